# revision 5
# baseline (speedup 1.0000x reference)
"""Trainium2 Bass kernel for nn_CP2_17669495456475 (dynamic-kernel deconv).

Math: out[b,c,y,x] = sum_l cos[b,l,i,j] * W[b,l,c,ky,kx],  y=8i+ky, x=8j+kx,
with W = unfold(pad(b)) * (1 - unfold(pad(mask))), K=16, S=8, crop 4.

Decomposition (per core): since K = 2*S, split ky = ry + 8*sy, kx = rx + 8*sx.
With u = i+sy, v = j+sx the whole op is ONE matmul with contraction over
(a,sy,sx,p) -> (l,sy,sx) of size 4096:

  outT[(c,ry,rx), (u,v)] = sum_{l,sy,sx} bm_block[(li+sy, lj+sx), (c,ry,rx)]
                                          * Xp[l, 1+u-sy, 1+v-sx]

where bm = pad(b)*(1-pad(mask)) laid out in 8x8 blocks (the unfold becomes
duplication-free shifted block views) and the deconv overlap-add is absorbed
into PSUM accumulation.  The mask multiply is fused on-device (DVE) on the
streamed W chunk tiles.

Sharding: 8 cores = 4 batches x 2 channel-halves (16 ch each). Full inputs in,
full output out; host does layout glue (replicate pad, block reshape, zero pad,
dtype cast, final crop/assembly) only.

Perf notes (v3): the warm MM stream runs at ~152-159ns per MM with no stalls;
all remaining time is startup, the cold-clock (HAM K=4/8) window, and the
tail.  Key facts measured on HW here: each HWDGE dma_start costs ~0.62us of
trigger emission on its engine, sub-1KB-per-partition transfers run at
~30-100GB/s (descriptor-dominated), and a DMA's semaphore fires ~0.4-1us
after its last byte.  So:
 - ONE packed critical DMA (mask cols 0-3 + all of chunk-0's W, 2.5KB/part
   contiguous) unlocks the first matmuls ~10.2us;
 - W streams as chunk1, chunk pair 2-3, then whole quads, in exact
   consumption order on the sync queue; X/masks stream on the scalar queue;
 - ~24 dependency-free warm-up matmuls on memset data keep the PE busy from
   ~7.8us so the HAM clock gate opens just as the real stream begins;
 - masks in bf16, 1-m and mask-muls sliced so the critical-path DVE work
   before the first matmul is ~300ns;
 - border chunks whose rhs row is the all-zero X pad (sy=1 in phase 0, sy=0
   in phase 2) run trimmed 10-row matmuls (N=330); phase-2 chunk order is
   rotated so a full-width chunk opens each accumulation group;
 - phases 1+2 fused m-outer/chunk-mid/n-inner: consecutive matmuls share
   lhsT (measured -4ns/MM); the last m runs n=1 then n=2 so the first
   half's writeback hides under the second half's matmuls;
 - bf16 output tiles; the final writeback is split into halves across both
   engines (copy) and both HWDGE queues (DMA) to shorten the tail.
"""

import os
import numpy as np
import ml_dtypes

import concourse.bass as bass
import concourse.mybir as mybir
import concourse.tile as tile
from concourse.bass_utils import run_bass_kernel_spmd

PD = 4
C = 16              # channels per core
N_CORES = 8
CHUNKS = [(a, sy, sx) for a in range(8) for sy in (0, 1) for sx in (0, 1)]

NV = 33
NT = 11 * NV          # N per matmul: 11 u-rows x NV v-cols

TRIM = os.environ.get("BASSK_TRIM", "1") == "1"
PAIR = os.environ.get("BASSK_PAIR", "1") == "1"
WARM = int(os.environ.get("BASSK_WARM", "26"))


def _split_multi_sync(nc):
    """The walrus in this env allows only ONE sync-wait per instruction.
    Hoist extra waits onto same-engine InstNoOp carriers placed just before
    the owning instruction (sequential waits on one engine == AND)."""
    ctr = 0
    for f in nc.m.functions:
        for bb in f.blocks:
            insts = list(bb.instructions)
            out = []
            changed = False
            for inst in insts:
                si = inst.sync_info
                waits = list(si.on_wait) if si and si.on_wait else []
                if len(waits) > 1:
                    for w in waits[:-1]:
                        nop = mybir.InstNoOp(name=f"waitnop-{ctr}", ins=[], outs=[])
                        ctr += 1
                        nop.engine = inst.engine
                        nop.sync_info = mybir.SyncInfo(on_wait=[w], on_update=[])
                        out.append(nop)
                    si.on_wait = [waits[-1]]
                    changed = True
                out.append(inst)
            if changed:
                bb.instructions = out
    return ctr


def _mm_rows(n, sy):
    """(row_lo, row_hi) within the 11-row phase window; trims the row that
    reads the all-zero X padding (u'=0 when n==0,sy==1; u'=10 when
    n==2,sy==0)."""
    lo, hi = 0, 11
    if TRIM and n == 0 and sy == 1:
        lo = 1
    if TRIM and n == 2 and sy == 0:
        hi = 10
    return lo, hi


def _build_nc():
    f32 = mybir.dt.float32
    bf16 = mybir.dt.bfloat16
    nc = bass.Bass(enable_partition_id=False)
    # Packed startup-critical tensor: [mask cols 0-3 | chunk-0 W | X a=0
    # slab-0 rows], 3.5KB/part — ONE well-shaped transfer on an otherwise
    # idle DMA fabric unlocks the whole first chunk.
    crit0 = nc.declare_dram_parameter("crit0", [128, 256 + C * 64 + 468], bf16,
                                      isOutput=False)
    # W chunks pre-gathered host-side, partition-major: [p, ci, (c,ry,rx)].
    w4 = nc.declare_dram_parameter("w4", [128, 32, C * 64], bf16, isOutput=False)
    mT = nc.declare_dram_parameter("mT", [128, 32, 64], bf16, isOutput=False)
    # X is y-major [p, yy, a, xx] and loads in 3 phase-aligned y-slabs:
    # phase n only reads rows [11n, 11n+13).
    xp = nc.declare_dram_parameter("xp", [128, 34, 8, 36], bf16, isOutput=False)
    # a-major copy of slab-0 rows so the startup pieces are contiguous
    xp0 = nc.declare_dram_parameter("xp0", [128, 8, 13, 36], bf16, isOutput=False)
    # out: phase-major [n, p, m, NT]
    outT = nc.declare_dram_parameter("outT", [3, 128, 8, NT], bf16, isOutput=True)

    with tile.TileContext(nc) as tc:
        with (
            tc.tile_pool(name="dzp", bufs=1) as dzp,
            tc.tile_pool(name="xpp", bufs=1) as xpp,
            tc.tile_pool(name="wp", bufs=1) as wp,
            tc.tile_pool(name="mp", bufs=1) as mp,
            tc.tile_pool(name="op", bufs=6) as op,
            tc.tile_pool(name="pp", bufs=8, space="PSUM") as pp,
        ):
            # --- PE warm-up bridge: the HAM clock gate opens only after
            # ~3.4us of sustained PE activity.  These matmuls depend only on
            # a local memset, so they start as soon as the engines clear the
            # preamble (~7.8us) and keep the PE busy until the first real MM
            # (~10.2us); the busy window then spans the transition and the
            # 2.4GHz clock arrives ~3.4us after the dummies began.  warm_ps
            # is slot 0 of the "ps" ring and doubles as phase-0 m=0's bank
            # (the real group's start=True matmul clears it).
            warm_ps = pp.tile([128, NT], f32, tag="ps", name="warm_ps")
            if WARM:
                dz = dzp.tile([128, 128], bf16, name="dz")
                nc.gpsimd.memset(dz[:], 0)
                for _ in range(WARM):
                    nc.tensor.matmul(warm_ps[:, 0:128], dz[:], dz[:],
                                     start=True, stop=True)

            # --- DMA plan.  sync queue: crit0, then W in consumption order.
            # scalar queue: X slab-0 piece, remaining masks, rest of slab 0,
            # deferred slabs 1-2, outT writebacks.
            cr = wp.tile([128, 256 + C * 64 + 468], bf16, name="cr")
            nc.sync.dma_start(cr[:], crit0[:])
            c1 = wp.tile([128, C * 64], bf16, name="c1")
            nc.sync.dma_start(c1[:], w4[:, 1])
            c23 = wp.tile([128, 2, C * 64], bf16, name="c23")
            nc.sync.dma_start(c23[:], w4[:, 2:4])
            halves = {}
            for h in range(4, 12, 2):       # chunk pairs 45, 67, 89, 1011
                t = wp.tile([128, 2, C * 64], bf16, name=f"ch_{h}")
                nc.sync.dma_start(t[:], w4[:, h:h + 2])
                halves[h] = t
            quads = {}
            for g in range(3, 8):
                wq = wp.tile([128, 4, C * 64], bf16, name=f"wq_{g}")
                nc.sync.dma_start(wq[:], w4[:, 4 * g:4 * g + 4, :])
                quads[g] = wq

            # scalar-queue stream is deferred behind the first matmul so the
            # critical sync-queue DMA gets the whole fabric during ramp
            mrest = mp.tile([128, 28, 64], bf16, name="mrest")
            mrest_dma = nc.scalar.dma_start(mrest[:], mT[:, 4:32])
            s0bc = xpp.tile([128, 7, 13, 36], bf16, name="s0bc")
            nc.scalar.dma_start(s0bc[:], xp0[:, 1:8])

            # DVE FIFO order matters: critical ops first, then per-chunk
            # mask-muls in consumption order, with each 1-m just ahead of
            # its first use.
            crm = cr[:, 0:256].rearrange("p (c f) -> p c f", c=4)
            nc.vector.tensor_scalar(
                out=crm, in0=crm, scalar1=-1.0, scalar2=1.0,
                op0=mybir.AluOpType.mult, op1=mybir.AluOpType.add,
            )

            def mask_ap(ci):
                if ci < 4:
                    return cr[:, 64 * ci:64 * (ci + 1)]
                return mrest[:, ci - 4, :]

            def mask_mul(w_ap, ci, nch):
                wv = w_ap.rearrange("p (c f) -> p c f", c=nch)
                nc.vector.tensor_tensor(
                    out=wv, in0=wv,
                    in1=mask_ap(ci)[:, None, :].broadcast_to([128, nch, 64]),
                    op=mybir.AluOpType.mult,
                )

            mask_mul(cr[:, 256:384], 0, 2)           # m=0 slice -> first MM
            mask_mul(cr[:, 384:768], 0, 6)           # m=1..3
            mask_mul(cr[:, 768:1280], 0, 8)          # m=4..7
            mask_mul(c1[:], 1, C)
            mask_mul(c23[:, 0, :], 2, C)
            mask_mul(c23[:, 1, :], 3, C)
            nc.vector.tensor_scalar(
                out=mrest[:], in0=mrest[:], scalar1=-1.0, scalar2=1.0,
                op0=mybir.AluOpType.mult, op1=mybir.AluOpType.add,
            )
            for ci in range(4, 12):
                mask_mul(halves[ci - ci % 2][:, ci % 2, :], ci, C)
            for ci in range(12, 32):
                mask_mul(quads[ci // 4][:, ci % 4, :], ci, C)

            def lhsT(ci, m):
                if ci == 0:
                    return cr[:, 256 + 128 * m:384 + 128 * m]
                if ci == 1:
                    return c1[:, 128 * m:128 * (m + 1)]
                if ci < 4:
                    return c23[:, ci - 2, 128 * m:128 * (m + 1)]
                if ci < 12:
                    h = ci - ci % 2
                    return halves[h][:, ci % 2, 128 * m:128 * (m + 1)]
                return quads[ci // 4][:, ci % 4, 128 * m:128 * (m + 1)]

            SLABS = ((0, 13), (11, 24), (22, 34))
            slab_dmas = [None]
            slabs = [None]
            for si, (y0, y1) in list(enumerate(SLABS))[1:]:
                st = xpp.tile([128, y1 - y0, 8, 36], bf16, name=f"slab_{si}")
                slab_dmas.append(nc.scalar.dma_start(st[:], xp[:, y0:y1]))
                slabs.append(st)

            def rhs_ap(n, ci):
                a, sy, sx = CHUNKS[ci]
                lo, hi = _mm_rows(n, sy)
                y0 = 11 * n + 1 - sy - SLABS[n][0]
                x0 = 1 - sx
                if n == 0:
                    if a == 0:
                        xv = cr[:, 1280:1748].rearrange(
                            "p (y x) -> p y x", y=13)
                        return xv[:, y0 + lo:y0 + hi, x0:x0 + NV]
                    return s0bc[:, a - 1, y0 + lo:y0 + hi, x0:x0 + NV]
                return slabs[n][:, y0 + lo:y0 + hi, a, x0:x0 + NV]

            def ps_ap(ps, n, ci):
                _, sy, _ = CHUNKS[ci]
                lo, hi = _mm_rows(n, sy)
                return ps[:, NV * lo:NV * hi]

            # Phase 2 accumulation order: a full-width (sy=1 when trimming)
            # chunk must open each group so the start=True matmul covers the
            # whole psum width.
            order2 = ([ci for ci in range(32) if CHUNKS[ci][1] == 1]
                      + [ci for ci in range(32) if CHUNKS[ci][1] == 0]) \
                if TRIM else list(range(32))

            # Phase 0 (n=0) is chunk-outer with 8 live psum groups so the PE
            # consumes W chunks as they stream (no all-32-chunks stall).
            pss = [warm_ps] + [pp.tile([128, NT], f32, tag="ps", name=f"ps_0_{i}")
                               for i in range(1, 8)]
            mm0 = {}
            for ci in range(32):
                rhs = rhs_ap(0, ci)
                for m in range(8):
                    mm0[ci, m] = nc.tensor.matmul(
                        ps_ap(pss[m], 0, ci), lhsT(ci, m), rhs,
                        start=(ci == 0), stop=(ci == 31),
                    )
            # Defer the X slabs for phases 1-2 behind PE progress so the
            # startup-critical stream gets the full DMA bandwidth during ramp.
            from concourse.tile_rust import add_dep_helper
            add_dep_helper(mrest_dma.ins, mm0[0, 0].ins,
                           sync=True, reason="scalar stream behind first MM")
            add_dep_helper(slab_dmas[1].ins, mm0[19, 7].ins,
                           sync=True, reason="slab1 after early phase0")
            add_dep_helper(slab_dmas[2].ins, mm0[27, 7].ins,
                           sync=True, reason="slab2 after mid phase0")
            # per-2m writeback tiles: whole-tile dependency tracking means one
            # big tile would hold every outT DMA until the LAST psum copy.
            for k in range(4):
                ot = op.tile([128, 2, NT], bf16, tag="o", name=f"osb_0_{k}")
                nc.vector.tensor_copy(ot[:, 0, :], pss[2 * k][:])
                nc.vector.tensor_copy(ot[:, 1, :], pss[2 * k + 1][:])
                nc.scalar.dma_start(outT[0, :, 2 * k:2 * k + 2], ot[:])

            def emit_12(m, paired):
                ps1 = pp.tile([128, NT], f32, tag="ps", name=f"ps_12_{m}a")
                ps2 = pp.tile([128, NT], f32, tag="ps", name=f"ps_12_{m}b")
                if paired:
                    for idx, ci in enumerate(order2):
                        st, sp = (idx == 0), (idx == 31)
                        w = lhsT(ci, m)
                        nc.tensor.matmul(ps_ap(ps1, 1, ci), w, rhs_ap(1, ci),
                                         start=st, stop=sp)
                        nc.tensor.matmul(ps_ap(ps2, 2, ci), w, rhs_ap(2, ci),
                                         start=st, stop=sp)
                else:
                    for n, ps in ((1, ps1), (2, ps2)):
                        for idx, ci in enumerate(order2):
                            nc.tensor.matmul(
                                ps_ap(ps, n, ci), lhsT(ci, m), rhs_ap(n, ci),
                                start=(idx == 0), stop=(idx == 31))
                return ps1, ps2

            if PAIR:
                # Phases 1+2 fused: m-outer, chunk-mid, n-inner.  The two
                # matmuls of a chunk share lhsT (stationary operand stays
                # loaded) and land in two psum banks.  All X is resident.
                # The last m runs n=1 fully, then n=2, so the n=1 writeback
                # hides under the n=2 matmuls and only the final half-group
                # writeback (split across engines and queues) is exposed.
                for m in range(7):
                    ps1, ps2 = emit_12(m, True)
                    ot = op.tile([128, 2, NT], bf16, tag="o", name=f"osb_12_{m}")
                    nc.vector.tensor_copy(ot[:, 0, :], ps1[:])
                    nc.vector.tensor_copy(ot[:, 1, :], ps2[:])
                    nc.scalar.dma_start(
                        outT[1:3, :, m:m + 1].rearrange("n p o f -> p (n o) f"),
                        ot[:])
                ps1, ps2 = emit_12(7, False)
                o1 = op.tile([128, 1, NT], bf16, tag="o", name="osb_t1")
                nc.vector.tensor_copy(o1[:, 0, :], ps1[:])
                nc.scalar.dma_start(outT[1, :, 7:8], o1[:])
                HNT = NT // 2
                o2 = op.tile([128, 1, NT], bf16, tag="o", name="osb_t2")
                nc.vector.tensor_copy(o2[:, 0, :], ps2[:])
                nc.scalar.dma_start(outT[2, :, 7, 0:HNT], o2[:, 0, 0:HNT])
                nc.sync.dma_start(outT[2, :, 7, HNT:NT], o2[:, 0, HNT:NT])
            else:
                for n in (1, 2):
                    order = order2 if n == 2 else list(range(32))
                    ot = None
                    for m in range(8):
                        ps = pp.tile([128, NT], f32, tag="ps", name=f"ps_{n}_{m}")
                        for idx, ci in enumerate(order):
                            nc.tensor.matmul(
                                ps_ap(ps, n, ci), lhsT(ci, m), rhs_ap(n, ci),
                                start=(idx == 0), stop=(idx == 31),
                            )
                        if n == 2 and m >= 6:
                            o1 = op.tile([128, 1, NT], bf16, tag="o",
                                         name=f"osb_t{m}")
                            nc.vector.tensor_copy(o1[:, 0, :], ps[:])
                            eng = nc.scalar if m == 6 else nc.sync
                            eng.dma_start(outT[n, :, m:m + 1], o1[:])
                        else:
                            if m % 2 == 0:
                                ot = op.tile([128, 2, NT], bf16, tag="o",
                                             name=f"osb_{n}_{m // 2}")
                            nc.vector.tensor_copy(ot[:, m % 2, :], ps[:])
                            if m % 2 == 1:
                                nc.scalar.dma_start(
                                    outT[n, :, m - 1:m + 1], ot[:])

    _split_multi_sync(nc)
    return nc


def _host_prep(b_ch, mask_b, cos_b):
    """b_ch (16,256,256) f32, mask_b (256,256) f32, cos_b (1024,32,32) f32
    -> dict of device inputs (layout/gather glue only)."""
    bpad = np.pad(b_ch, ((0, 0), (PD, PD), (PD, PD)), mode="edge")
    mpad = np.pad(mask_b, ((PD, PD), (PD, PD)), mode="edge")
    # block layout [bi*33+bj, (c,ry,rx)]
    bT = bpad.reshape(C, 33, 8, 33, 8).transpose(1, 3, 0, 2, 4).reshape(33 * 33, C * 64)
    mTb = mpad.reshape(33, 8, 33, 8).transpose(0, 2, 1, 3).reshape(33 * 33, 64)
    # unfold-as-shifted-block-views: chunk (a,sy,sx), partition p=32*pi+pj
    # reads block row (4a+pi+sy)*33 + (pj+sx).  Pre-gather partition-major.
    pi, pj = np.arange(4)[:, None], np.arange(32)[None, :]
    rows = np.stack([((4 * a + pi + sy) * 33 + (pj + sx)).reshape(128)
                     for (a, sy, sx) in CHUNKS], axis=1)        # [128, 32]
    w4 = np.ascontiguousarray(bT[rows]).astype(ml_dtypes.bfloat16)
    mT = np.ascontiguousarray(mTb[rows]).astype(ml_dtypes.bfloat16)
    xp = np.zeros((1024, 34, 36), np.float32)
    xp[:, 1:33, 1:33] = cos_b
    # [l=128a+p, yy, xx] -> [p, yy, a, xx]; plus an a-major slab-0 copy
    xpb = xp.reshape(8, 128, 34, 36)
    xp0 = np.ascontiguousarray(xpb[:, :, 0:13, :].transpose(1, 0, 2, 3))
    crit0 = np.ascontiguousarray(np.concatenate(
        [mT[:, 0:4].reshape(128, 256), w4[:, 0, :],
         xp0[:, 0].reshape(128, 468).astype(ml_dtypes.bfloat16)], axis=1))
    xp = np.ascontiguousarray(xpb.transpose(1, 2, 0, 3))
    return {"crit0": crit0, "w4": w4, "mT": mT,
            "xp": xp.astype(ml_dtypes.bfloat16),
            "xp0": xp0.astype(ml_dtypes.bfloat16)}


def _unshard(outT):
    # outT [3, 128, 8, 11*NV] -> [(c,ry,rx)=128m+p, u=11n+u', v] -> (16,256,256)
    outT = np.asarray(outT).astype(np.float32)
    t = outT.reshape(3, 128, 8, 11, NV).transpose(2, 1, 0, 3, 4).reshape(1024, 33, NV)
    t = t[:, :, :33].reshape(C, 8, 8, 33, 33).transpose(0, 3, 1, 4, 2)
    return t.reshape(C, 264, 264)[:, 4:260, 4:260]


_RUN_KW = {}   # test harness may inject e.g. trace=True
_LAST_RESULTS = [None]
_NC_CACHE = {}


def _get_nc():
    key = (TRIM, PAIR, WARM)
    nc = _NC_CACHE.get(key)
    if nc is None:
        nc = _NC_CACHE[key] = _build_nc()
    return nc


def kernel(cos_similar, b, mask):
    cos_similar = np.ascontiguousarray(np.asarray(cos_similar, dtype=np.float32))
    b = np.ascontiguousarray(np.asarray(b, dtype=np.float32))
    mask = np.ascontiguousarray(np.asarray(mask, dtype=np.float32))

    in_maps = []
    for core in range(N_CORES):
        batch, half = core // 2, core % 2
        ch0 = C * half
        in_maps.append(_host_prep(
            b[batch, ch0:ch0 + C], mask[batch, 0], cos_similar[batch]))

    nc = _get_nc()
    res = run_bass_kernel_spmd(nc, in_maps, list(range(N_CORES)), **_RUN_KW)
    _LAST_RESULTS[0] = res

    out = np.empty((4, 32, 256, 256), np.float32)
    for core in range(N_CORES):
        batch, half = core // 2, core % 2
        ch0 = C * half
        out[batch, ch0:ch0 + C] = _unshard(res.results[core]["outT"])
    return out


# revision 6
# speedup vs baseline: 1.0220x; 1.0220x over previous
"""Trainium2 Bass kernel for nn_CP2_17669495456475 (dynamic-kernel deconv).

Math: out[b,c,y,x] = sum_l cos[b,l,i,j] * W[b,l,c,ky,kx],  y=8i+ky, x=8j+kx,
with W = unfold(pad(b)) * (1 - unfold(pad(mask))), K=16, S=8, crop 4.

Decomposition (per core): since K = 2*S, split ky = ry + 8*sy, kx = rx + 8*sx.
With u = i+sy, v = j+sx the whole op is ONE matmul with contraction over
(a,sy,sx,p) -> (l,sy,sx) of size 4096:

  outT[(c,ry,rx), (u,v)] = sum_{l,sy,sx} bm_block[(li+sy, lj+sx), (c,ry,rx)]
                                          * Xp[l, 1+u-sy, 1+v-sx]

where bm = pad(b)*(1-pad(mask)) laid out in 8x8 blocks (the unfold becomes
duplication-free shifted block views) and the deconv overlap-add is absorbed
into PSUM accumulation.  The mask multiply is fused on-device (DVE) on the
streamed W chunk tiles.

Sharding: 8 cores = 4 batches x 2 channel-halves (16 ch each). Full inputs in,
full output out; host does layout glue (replicate pad, block reshape, zero pad,
dtype cast, final crop/assembly) only.

Perf notes (v3): the warm MM stream runs at ~152-159ns per MM with no stalls;
all remaining time is startup, the cold-clock (HAM K=4/8) window, and the
tail.  Key facts measured on HW here: each HWDGE dma_start costs ~0.62us of
trigger emission on its engine, sub-1KB-per-partition transfers run at
~30-100GB/s (descriptor-dominated), and a DMA's semaphore fires ~0.4-1us
after its last byte.  So:
 - ONE packed critical DMA (mask cols 0-3 + all of chunk-0's W, 2.5KB/part
   contiguous) unlocks the first matmuls ~10.2us;
 - W streams as chunk1, chunk pair 2-3, then whole quads, in exact
   consumption order on the sync queue; X/masks stream on the scalar queue;
 - ~24 dependency-free warm-up matmuls on memset data keep the PE busy from
   ~7.8us so the HAM clock gate opens just as the real stream begins;
 - masks in bf16, 1-m and mask-muls sliced so the critical-path DVE work
   before the first matmul is ~300ns;
 - border chunks whose rhs row is the all-zero X pad (sy=1 in phase 0, sy=0
   in phase 2) run trimmed 10-row matmuls (N=330); phase-2 chunk order is
   rotated so a full-width chunk opens each accumulation group;
 - phases 1+2 fused m-outer/chunk-mid/n-inner: consecutive matmuls share
   lhsT (measured -4ns/MM); the last m runs n=1 then n=2 so the first
   half's writeback hides under the second half's matmuls;
 - bf16 output tiles; the final writeback is split into halves across both
   engines (copy) and both HWDGE queues (DMA) to shorten the tail.
"""

import os
import numpy as np
import ml_dtypes

import concourse.bass as bass
import concourse.mybir as mybir
import concourse.tile as tile
from concourse.bass_utils import run_bass_kernel_spmd

PD = 4
C = 16              # channels per core
N_CORES = 8
CHUNKS = [(a, sy, sx) for a in range(8) for sy in (0, 1) for sx in (0, 1)]

NV = 33
NT = 11 * NV          # N per matmul: 11 u-rows x NV v-cols

TRIM = os.environ.get("BASSK_TRIM", "1") == "1"
PAIR = os.environ.get("BASSK_PAIR", "1") == "1"
WARM = int(os.environ.get("BASSK_WARM", "38"))


def _split_multi_sync(nc):
    """The walrus in this env allows only ONE sync-wait per instruction.
    Hoist extra waits onto same-engine InstNoOp carriers placed just before
    the owning instruction (sequential waits on one engine == AND)."""
    ctr = 0
    for f in nc.m.functions:
        for bb in f.blocks:
            insts = list(bb.instructions)
            out = []
            changed = False
            for inst in insts:
                si = inst.sync_info
                waits = list(si.on_wait) if si and si.on_wait else []
                if len(waits) > 1:
                    for w in waits[:-1]:
                        nop = mybir.InstNoOp(name=f"waitnop-{ctr}", ins=[], outs=[])
                        ctr += 1
                        nop.engine = inst.engine
                        nop.sync_info = mybir.SyncInfo(on_wait=[w], on_update=[])
                        out.append(nop)
                    si.on_wait = [waits[-1]]
                    changed = True
                out.append(inst)
            if changed:
                bb.instructions = out
    return ctr


def _mm_rows(n, sy):
    """(row_lo, row_hi) within the 11-row phase window; trims the row that
    reads the all-zero X padding (u'=0 when n==0,sy==1; u'=10 when
    n==2,sy==0)."""
    lo, hi = 0, 11
    if TRIM and n == 0 and sy == 1:
        lo = 1
    if TRIM and n == 2 and sy == 0:
        hi = 10
    return lo, hi


def _build_nc():
    f32 = mybir.dt.float32
    bf16 = mybir.dt.bfloat16
    nc = bass.Bass(enable_partition_id=False)
    # Packed startup-critical tensor: [mask cols 0-3 | chunk-0 W],
    # 2.5KB/part — one well-shaped transfer unlocks the whole first chunk.
    crit0 = nc.declare_dram_parameter("crit0", [128, 256 + C * 64], bf16,
                                      isOutput=False)
    # W chunks pre-gathered host-side, partition-major: [p, ci, (c,ry,rx)].
    w4 = nc.declare_dram_parameter("w4", [128, 32, C * 64], bf16, isOutput=False)
    mT = nc.declare_dram_parameter("mT", [128, 32, 64], bf16, isOutput=False)
    # X is y-major [p, yy, a, xx] and loads in 3 phase-aligned y-slabs:
    # phase n only reads rows [11n, 11n+13).
    xp = nc.declare_dram_parameter("xp", [128, 34, 8, 36], bf16, isOutput=False)
    # a-major copy of slab-0 rows so the startup pieces are contiguous
    xp0 = nc.declare_dram_parameter("xp0", [128, 8, 13, 36], bf16, isOutput=False)
    # out: phase-major [n, p, m, NT]
    outT = nc.declare_dram_parameter("outT", [3, 128, 8, NT], bf16, isOutput=True)

    with tile.TileContext(nc) as tc:
        with (
            tc.tile_pool(name="dzp", bufs=1) as dzp,
            tc.tile_pool(name="xpp", bufs=1) as xpp,
            tc.tile_pool(name="wp", bufs=1) as wp,
            tc.tile_pool(name="mp", bufs=1) as mp,
            tc.tile_pool(name="op", bufs=6) as op,
            tc.tile_pool(name="pp", bufs=8, space="PSUM") as pp,
        ):
            # --- PE warm-up bridge: the HAM clock gate opens only after
            # ~3.4us of sustained PE activity.  These matmuls depend only on
            # a local memset, so they start as soon as the engines clear the
            # preamble (~7.8us) and keep the PE busy until the first real MM
            # (~10.2us); the busy window then spans the transition and the
            # 2.4GHz clock arrives ~3.4us after the dummies began.  warm_ps
            # is slot 0 of the "ps" ring and doubles as phase-0 m=0's bank
            # (the real group's start=True matmul clears it).
            warm_ps = pp.tile([128, NT], f32, tag="ps", name="warm_ps")
            if WARM:
                dz = dzp.tile([128, 128], bf16, name="dz")
                nc.gpsimd.memset(dz[:], 0)
                for _ in range(WARM):
                    nc.tensor.matmul(warm_ps[:, 0:128], dz[:], dz[:],
                                     start=True, stop=True)

            # --- DMA plan.  sync queue: crit0, then W in consumption order.
            # scalar queue: X slab-0 piece, remaining masks, rest of slab 0,
            # deferred slabs 1-2, outT writebacks.
            cr = wp.tile([128, 256 + C * 64], bf16, name="cr")
            nc.sync.dma_start(cr[:], crit0[:])
            c1 = wp.tile([128, C * 64], bf16, name="c1")
            nc.sync.dma_start(c1[:], w4[:, 1])
            c23 = wp.tile([128, 2, C * 64], bf16, name="c23")
            nc.sync.dma_start(c23[:], w4[:, 2:4])
            halves = {}
            for h in range(4, 12, 2):       # chunk pairs 45, 67, 89, 1011
                t = wp.tile([128, 2, C * 64], bf16, name=f"ch_{h}")
                nc.sync.dma_start(t[:], w4[:, h:h + 2])
                halves[h] = t
            quads = {}
            for g in range(3, 8):
                wq = wp.tile([128, 4, C * 64], bf16, name=f"wq_{g}")
                nc.sync.dma_start(wq[:], w4[:, 4 * g:4 * g + 4, :])
                quads[g] = wq

            # scalar queue carries X and the remaining masks in global
            # just-in-time order, in parallel with the sync queue's W stream
            s0a = xpp.tile([128, 1, 13, 36], bf16, name="s0a")
            nc.scalar.dma_start(s0a[:], xp0[:, 0:1])
            mrest = mp.tile([128, 28, 64], bf16, name="mrest")
            nc.scalar.dma_start(mrest[:], mT[:, 4:32])
            s0b1 = xpp.tile([128, 1, 13, 36], bf16, name="s0b1")
            nc.scalar.dma_start(s0b1[:], xp0[:, 1:2])
            s0cd = xpp.tile([128, 6, 13, 36], bf16, name="s0cd")
            nc.scalar.dma_start(s0cd[:], xp0[:, 2:8])

            # DVE FIFO order matters: critical ops first, then per-chunk
            # mask-muls in consumption order, with each 1-m just ahead of
            # its first use.
            crm = cr[:, 0:256].rearrange("p (c f) -> p c f", c=4)
            nc.vector.tensor_scalar(
                out=crm, in0=crm, scalar1=-1.0, scalar2=1.0,
                op0=mybir.AluOpType.mult, op1=mybir.AluOpType.add,
            )

            def mask_ap(ci):
                if ci < 4:
                    return cr[:, 64 * ci:64 * (ci + 1)]
                return mrest[:, ci - 4, :]

            def mask_mul(w_ap, ci, nch):
                wv = w_ap.rearrange("p (c f) -> p c f", c=nch)
                nc.vector.tensor_tensor(
                    out=wv, in0=wv,
                    in1=mask_ap(ci)[:, None, :].broadcast_to([128, nch, 64]),
                    op=mybir.AluOpType.mult,
                )

            mask_mul(cr[:, 256:384], 0, 2)           # m=0 slice -> first MM
            mask_mul(cr[:, 384:768], 0, 6)           # m=1..3
            mask_mul(cr[:, 768:1280], 0, 8)          # m=4..7
            mask_mul(c1[:], 1, C)
            mask_mul(c23[:, 0, :], 2, C)
            mask_mul(c23[:, 1, :], 3, C)
            nc.vector.tensor_scalar(
                out=mrest[:], in0=mrest[:], scalar1=-1.0, scalar2=1.0,
                op0=mybir.AluOpType.mult, op1=mybir.AluOpType.add,
            )
            for ci in range(4, 12):
                mask_mul(halves[ci - ci % 2][:, ci % 2, :], ci, C)
            for ci in range(12, 32):
                mask_mul(quads[ci // 4][:, ci % 4, :], ci, C)

            def lhsT(ci, m):
                if ci == 0:
                    return cr[:, 256 + 128 * m:384 + 128 * m]
                if ci == 1:
                    return c1[:, 128 * m:128 * (m + 1)]
                if ci < 4:
                    return c23[:, ci - 2, 128 * m:128 * (m + 1)]
                if ci < 12:
                    h = ci - ci % 2
                    return halves[h][:, ci % 2, 128 * m:128 * (m + 1)]
                return quads[ci // 4][:, ci % 4, 128 * m:128 * (m + 1)]

            SLABS = ((0, 13), (11, 24), (22, 34))
            slab_dmas = [None]
            slabs = [None]
            for si, (y0, y1) in list(enumerate(SLABS))[1:]:
                st = xpp.tile([128, y1 - y0, 8, 36], bf16, name=f"slab_{si}")
                slab_dmas.append(nc.scalar.dma_start(st[:], xp[:, y0:y1]))
                slabs.append(st)

            def rhs_ap(n, ci):
                a, sy, sx = CHUNKS[ci]
                lo, hi = _mm_rows(n, sy)
                y0 = 11 * n + 1 - sy - SLABS[n][0]
                x0 = 1 - sx
                if n == 0:
                    t, aa = ((s0a, 0) if a == 0 else
                             (s0b1, 0) if a == 1 else (s0cd, a - 2))
                    return t[:, aa, y0 + lo:y0 + hi, x0:x0 + NV]
                return slabs[n][:, y0 + lo:y0 + hi, a, x0:x0 + NV]

            def ps_ap(ps, n, ci):
                _, sy, _ = CHUNKS[ci]
                lo, hi = _mm_rows(n, sy)
                return ps[:, NV * lo:NV * hi]

            # Phase 2 accumulation order: a full-width (sy=1 when trimming)
            # chunk must open each group so the start=True matmul covers the
            # whole psum width.
            order2 = ([ci for ci in range(32) if CHUNKS[ci][1] == 1]
                      + [ci for ci in range(32) if CHUNKS[ci][1] == 0]) \
                if TRIM else list(range(32))

            # Phase 0 (n=0) is chunk-outer with 8 live psum groups so the PE
            # consumes W chunks as they stream (no all-32-chunks stall).
            pss = [warm_ps] + [pp.tile([128, NT], f32, tag="ps", name=f"ps_0_{i}")
                               for i in range(1, 8)]
            mm0 = {}
            for ci in range(32):
                rhs = rhs_ap(0, ci)
                for m in range(8):
                    mm0[ci, m] = nc.tensor.matmul(
                        ps_ap(pss[m], 0, ci), lhsT(ci, m), rhs,
                        start=(ci == 0), stop=(ci == 31),
                    )
            # Defer the X slabs for phases 1-2 behind PE progress so the
            # startup-critical stream gets the full DMA bandwidth during ramp.
            from concourse.tile_rust import add_dep_helper
            add_dep_helper(slab_dmas[1].ins, mm0[15, 7].ins,
                           sync=True, reason="slab1 after early phase0")
            add_dep_helper(slab_dmas[2].ins, mm0[23, 7].ins,
                           sync=True, reason="slab2 after mid phase0")
            # per-2m writeback tiles: whole-tile dependency tracking means one
            # big tile would hold every outT DMA until the LAST psum copy.
            for k in range(4):
                ot = op.tile([128, 2, NT], bf16, tag="o", name=f"osb_0_{k}")
                nc.vector.tensor_copy(ot[:, 0, :], pss[2 * k][:])
                nc.vector.tensor_copy(ot[:, 1, :], pss[2 * k + 1][:])
                nc.scalar.dma_start(outT[0, :, 2 * k:2 * k + 2], ot[:])

            def emit_12(m, paired):
                ps1 = pp.tile([128, NT], f32, tag="ps", name=f"ps_12_{m}a")
                ps2 = pp.tile([128, NT], f32, tag="ps", name=f"ps_12_{m}b")
                if paired:
                    for idx, ci in enumerate(order2):
                        st, sp = (idx == 0), (idx == 31)
                        w = lhsT(ci, m)
                        nc.tensor.matmul(ps_ap(ps1, 1, ci), w, rhs_ap(1, ci),
                                         start=st, stop=sp)
                        nc.tensor.matmul(ps_ap(ps2, 2, ci), w, rhs_ap(2, ci),
                                         start=st, stop=sp)
                else:
                    for n, ps in ((1, ps1), (2, ps2)):
                        for idx, ci in enumerate(order2):
                            nc.tensor.matmul(
                                ps_ap(ps, n, ci), lhsT(ci, m), rhs_ap(n, ci),
                                start=(idx == 0), stop=(idx == 31))
                return ps1, ps2

            if PAIR:
                # Phases 1+2 fused: m-outer, chunk-mid, n-inner.  The two
                # matmuls of a chunk share lhsT (stationary operand stays
                # loaded) and land in two psum banks.  All X is resident.
                # The last m runs n=1 fully, then n=2, so the n=1 writeback
                # hides under the n=2 matmuls and only the final half-group
                # writeback (split across engines and queues) is exposed.
                for m in range(7):
                    ps1, ps2 = emit_12(m, True)
                    ot = op.tile([128, 2, NT], bf16, tag="o", name=f"osb_12_{m}")
                    nc.vector.tensor_copy(ot[:, 0, :], ps1[:])
                    nc.vector.tensor_copy(ot[:, 1, :], ps2[:])
                    nc.scalar.dma_start(
                        outT[1:3, :, m:m + 1].rearrange("n p o f -> p (n o) f"),
                        ot[:])
                ps1, ps2 = emit_12(7, False)
                o1 = op.tile([128, 1, NT], bf16, tag="o", name="osb_t1")
                nc.vector.tensor_copy(o1[:, 0, :], ps1[:])
                nc.scalar.dma_start(outT[1, :, 7:8], o1[:])
                HNT = NT // 2
                o2 = op.tile([128, 1, NT], bf16, tag="o", name="osb_t2")
                nc.vector.tensor_copy(o2[:, 0, :], ps2[:])
                nc.scalar.dma_start(outT[2, :, 7, 0:HNT], o2[:, 0, 0:HNT])
                nc.sync.dma_start(outT[2, :, 7, HNT:NT], o2[:, 0, HNT:NT])
            else:
                for n in (1, 2):
                    order = order2 if n == 2 else list(range(32))
                    ot = None
                    for m in range(8):
                        ps = pp.tile([128, NT], f32, tag="ps", name=f"ps_{n}_{m}")
                        for idx, ci in enumerate(order):
                            nc.tensor.matmul(
                                ps_ap(ps, n, ci), lhsT(ci, m), rhs_ap(n, ci),
                                start=(idx == 0), stop=(idx == 31),
                            )
                        if n == 2 and m >= 6:
                            o1 = op.tile([128, 1, NT], bf16, tag="o",
                                         name=f"osb_t{m}")
                            nc.vector.tensor_copy(o1[:, 0, :], ps[:])
                            eng = nc.scalar if m == 6 else nc.sync
                            eng.dma_start(outT[n, :, m:m + 1], o1[:])
                        else:
                            if m % 2 == 0:
                                ot = op.tile([128, 2, NT], bf16, tag="o",
                                             name=f"osb_{n}_{m // 2}")
                            nc.vector.tensor_copy(ot[:, m % 2, :], ps[:])
                            if m % 2 == 1:
                                nc.scalar.dma_start(
                                    outT[n, :, m - 1:m + 1], ot[:])

    _split_multi_sync(nc)
    return nc


def _host_prep(b_ch, mask_b, cos_b):
    """b_ch (16,256,256) f32, mask_b (256,256) f32, cos_b (1024,32,32) f32
    -> dict of device inputs (layout/gather glue only)."""
    bpad = np.pad(b_ch, ((0, 0), (PD, PD), (PD, PD)), mode="edge")
    mpad = np.pad(mask_b, ((PD, PD), (PD, PD)), mode="edge")
    # block layout [bi*33+bj, (c,ry,rx)]
    bT = bpad.reshape(C, 33, 8, 33, 8).transpose(1, 3, 0, 2, 4).reshape(33 * 33, C * 64)
    mTb = mpad.reshape(33, 8, 33, 8).transpose(0, 2, 1, 3).reshape(33 * 33, 64)
    # unfold-as-shifted-block-views: chunk (a,sy,sx), partition p=32*pi+pj
    # reads block row (4a+pi+sy)*33 + (pj+sx).  Pre-gather partition-major.
    pi, pj = np.arange(4)[:, None], np.arange(32)[None, :]
    rows = np.stack([((4 * a + pi + sy) * 33 + (pj + sx)).reshape(128)
                     for (a, sy, sx) in CHUNKS], axis=1)        # [128, 32]
    w4 = np.ascontiguousarray(bT[rows]).astype(ml_dtypes.bfloat16)
    mT = np.ascontiguousarray(mTb[rows]).astype(ml_dtypes.bfloat16)
    xp = np.zeros((1024, 34, 36), np.float32)
    xp[:, 1:33, 1:33] = cos_b
    # [l=128a+p, yy, xx] -> [p, yy, a, xx]; plus an a-major slab-0 copy
    xpb = xp.reshape(8, 128, 34, 36)
    xp0 = np.ascontiguousarray(xpb[:, :, 0:13, :].transpose(1, 0, 2, 3))
    crit0 = np.ascontiguousarray(np.concatenate(
        [mT[:, 0:4].reshape(128, 256), w4[:, 0, :]], axis=1))
    xp = np.ascontiguousarray(xpb.transpose(1, 2, 0, 3))
    return {"crit0": crit0, "w4": w4, "mT": mT,
            "xp": xp.astype(ml_dtypes.bfloat16),
            "xp0": xp0.astype(ml_dtypes.bfloat16)}


def _unshard(outT):
    # outT [3, 128, 8, 11*NV] -> [(c,ry,rx)=128m+p, u=11n+u', v] -> (16,256,256)
    outT = np.asarray(outT).astype(np.float32)
    t = outT.reshape(3, 128, 8, 11, NV).transpose(2, 1, 0, 3, 4).reshape(1024, 33, NV)
    t = t[:, :, :33].reshape(C, 8, 8, 33, 33).transpose(0, 3, 1, 4, 2)
    return t.reshape(C, 264, 264)[:, 4:260, 4:260]


_RUN_KW = {}   # test harness may inject e.g. trace=True
_LAST_RESULTS = [None]
_NC_CACHE = {}


def _get_nc():
    key = (TRIM, PAIR, WARM)
    nc = _NC_CACHE.get(key)
    if nc is None:
        nc = _NC_CACHE[key] = _build_nc()
    return nc


def kernel(cos_similar, b, mask):
    cos_similar = np.ascontiguousarray(np.asarray(cos_similar, dtype=np.float32))
    b = np.ascontiguousarray(np.asarray(b, dtype=np.float32))
    mask = np.ascontiguousarray(np.asarray(mask, dtype=np.float32))

    in_maps = []
    for core in range(N_CORES):
        batch, half = core // 2, core % 2
        ch0 = C * half
        in_maps.append(_host_prep(
            b[batch, ch0:ch0 + C], mask[batch, 0], cos_similar[batch]))

    nc = _get_nc()
    res = run_bass_kernel_spmd(nc, in_maps, list(range(N_CORES)), **_RUN_KW)
    _LAST_RESULTS[0] = res

    out = np.empty((4, 32, 256, 256), np.float32)
    for core in range(N_CORES):
        batch, half = core // 2, core % 2
        ch0 = C * half
        out[batch, ch0:ch0 + C] = _unshard(res.results[core]["outT"])
    return out


# revision 9
# speedup vs baseline: 1.0400x; 1.0177x over previous
"""Trainium2 Bass kernel for nn_CP2_17669495456475 (dynamic-kernel deconv).

Math: out[b,c,y,x] = sum_l cos[b,l,i,j] * W[b,l,c,ky,kx],  y=8i+ky, x=8j+kx,
with W = unfold(pad(b)) * (1 - unfold(pad(mask))), K=16, S=8, crop 4.

Decomposition (per core): since K = 2*S, split ky = ry + 8*sy, kx = rx + 8*sx.
With u = i+sy, v = j+sx the whole op is ONE matmul with contraction over
(a,sy,sx,p) -> (l,sy,sx) of size 4096:

  outT[(c,ry,rx), (u,v)] = sum_{l,sy,sx} bm_block[(li+sy, lj+sx), (c,ry,rx)]
                                          * Xp[l, 1+u-sy, 1+v-sx]

where bm = pad(b)*(1-pad(mask)) laid out in 8x8 blocks (the unfold becomes
duplication-free shifted block views) and the deconv overlap-add is absorbed
into PSUM accumulation.  The mask multiply is fused on-device (DVE) on the
streamed W chunk tiles.

Sharding: 8 cores = 4 batches x 2 channel-halves (16 ch each). Full inputs in,
full output out; host does layout glue (replicate pad, block reshape, zero pad,
dtype cast, final crop/assembly) only.

Perf notes (v3): the warm MM stream runs at ~152-159ns per MM with no stalls;
all remaining time is startup, the cold-clock (HAM K=4/8) window, and the
tail.  Key facts measured on HW here: each HWDGE dma_start costs ~0.62us of
trigger emission on its engine, sub-1KB-per-partition transfers run at
~30-100GB/s (descriptor-dominated), and a DMA's semaphore fires ~0.4-1us
after its last byte.  So:
 - ONE packed critical DMA (mask cols 0-3 + all of chunk-0's W, 2.5KB/part
   contiguous) unlocks the first matmuls ~10.2us;
 - W streams as chunk1, chunk pair 2-3, then whole quads, in exact
   consumption order on the sync queue; X/masks stream on the scalar queue;
 - ~24 dependency-free warm-up matmuls on memset data keep the PE busy from
   ~7.8us so the HAM clock gate opens just as the real stream begins;
 - masks in bf16, 1-m and mask-muls sliced so the critical-path DVE work
   before the first matmul is ~300ns;
 - border chunks whose rhs row is the all-zero X pad (sy=1 in phase 0, sy=0
   in phase 2) run trimmed 10-row matmuls (N=330); phase-2 chunk order is
   rotated so a full-width chunk opens each accumulation group;
 - phases 1+2 fused m-outer/chunk-mid/n-inner: consecutive matmuls share
   lhsT (measured -4ns/MM); the last m runs n=1 then n=2 so the first
   half's writeback hides under the second half's matmuls;
 - bf16 output tiles; the final writeback is split into halves across both
   engines (copy) and both HWDGE queues (DMA) to shorten the tail.
"""

import os
import numpy as np
import ml_dtypes

import concourse.bass as bass
import concourse.mybir as mybir
import concourse.tile as tile
from concourse.bass_utils import run_bass_kernel_spmd

PD = 4
C = 16              # channels per core
N_CORES = 8
CHUNKS = [(a, sy, sx) for a in range(8) for sy in (0, 1) for sx in (0, 1)]

NV = 33
NT = 11 * NV          # N per matmul: 11 u-rows x NV v-cols

TRIM = os.environ.get("BASSK_TRIM", "1") == "1"
PAIR = os.environ.get("BASSK_PAIR", "1") == "1"
WARM = int(os.environ.get("BASSK_WARM", "38"))


def _split_multi_sync(nc):
    """The walrus in this env allows only ONE sync-wait per instruction.
    Hoist extra waits onto same-engine InstNoOp carriers placed just before
    the owning instruction (sequential waits on one engine == AND)."""
    ctr = 0
    for f in nc.m.functions:
        for bb in f.blocks:
            insts = list(bb.instructions)
            out = []
            changed = False
            for inst in insts:
                si = inst.sync_info
                waits = list(si.on_wait) if si and si.on_wait else []
                if len(waits) > 1:
                    for w in waits[:-1]:
                        nop = mybir.InstNoOp(name=f"waitnop-{ctr}", ins=[], outs=[])
                        ctr += 1
                        nop.engine = inst.engine
                        nop.sync_info = mybir.SyncInfo(on_wait=[w], on_update=[])
                        out.append(nop)
                    si.on_wait = [waits[-1]]
                    changed = True
                out.append(inst)
            if changed:
                bb.instructions = out
    return ctr


def _mm_rows(n, sy):
    """(row_lo, row_hi) within the 11-row phase window; trims the row that
    reads the all-zero X padding (u'=0 when n==0,sy==1; u'=10 when
    n==2,sy==0)."""
    lo, hi = 0, 11
    if TRIM and n == 0 and sy == 1:
        lo = 1
    if TRIM and n == 2 and sy == 0:
        hi = 10
    return lo, hi


def _build_nc():
    f32 = mybir.dt.float32
    bf16 = mybir.dt.bfloat16
    nc = bass.Bass(enable_partition_id=False)
    # Packed startup-critical tensor: [mask cols 0-3 | chunk-0 W],
    # 2.5KB/part — one well-shaped transfer unlocks the whole first chunk.
    crit0 = nc.declare_dram_parameter("crit0", [128, 256 + C * 64], bf16,
                                      isOutput=False)
    # W chunks pre-gathered host-side, partition-major: [p, ci, (c,ry,rx)].
    w4 = nc.declare_dram_parameter("w4", [128, 32, C * 64], bf16, isOutput=False)
    mT = nc.declare_dram_parameter("mT", [128, 32, 64], bf16, isOutput=False)
    # X is y-major [p, yy, a, xx] and loads in 3 phase-aligned y-slabs:
    # phase n only reads rows [11n, 11n+13).
    xp = nc.declare_dram_parameter("xp", [128, 34, 8, 36], bf16, isOutput=False)
    # a-major copy of slab-0 rows so the startup pieces are contiguous
    xp0 = nc.declare_dram_parameter("xp0", [128, 8, 13, 36], bf16, isOutput=False)
    # out: phase-major [n, p, m, NT]
    outT = nc.declare_dram_parameter("outT", [3, 128, 8, NT], bf16, isOutput=True)

    with tile.TileContext(nc) as tc:
        with (
            tc.tile_pool(name="dzp", bufs=1) as dzp,
            tc.tile_pool(name="xpp", bufs=1) as xpp,
            tc.tile_pool(name="wp", bufs=1) as wp,
            tc.tile_pool(name="mp", bufs=1) as mp,
            tc.tile_pool(name="op", bufs=6) as op,
            tc.tile_pool(name="pp", bufs=8, space="PSUM") as pp,
        ):
            # --- PE warm-up bridge: the HAM clock gate opens only after
            # ~3.4us of sustained PE activity.  These matmuls depend only on
            # a local memset, so they start as soon as the engines clear the
            # preamble (~7.8us) and keep the PE busy until the first real MM
            # (~10.2us); the busy window then spans the transition and the
            # 2.4GHz clock arrives ~3.4us after the dummies began.  warm_ps
            # is slot 0 of the "ps" ring and doubles as phase-0 m=0's bank
            # (the real group's start=True matmul clears it).
            warm_ps = pp.tile([128, NT], f32, tag="ps", name="warm_ps")
            if WARM:
                dz = dzp.tile([128, 128], bf16, name="dz")
                nc.gpsimd.memset(dz[:], 0)
                for _ in range(WARM):
                    nc.tensor.matmul(warm_ps[:, 0:128], dz[:], dz[:],
                                     start=True, stop=True)

            # --- DMA plan.  sync queue: crit0, then W in consumption order.
            # scalar queue: X slab-0 piece, remaining masks, rest of slab 0,
            # deferred slabs 1-2, outT writebacks.
            cr = wp.tile([128, 256 + C * 64], bf16, name="cr")
            nc.sync.dma_start(cr[:], crit0[:])
            c1 = wp.tile([128, C * 64], bf16, name="c1")
            nc.sync.dma_start(c1[:], w4[:, 1])
            c23 = wp.tile([128, 2, C * 64], bf16, name="c23")
            nc.sync.dma_start(c23[:], w4[:, 2:4])
            halves = {}
            for h in range(4, 12, 2):       # chunk pairs 45, 67, 89, 1011
                t = wp.tile([128, 2, C * 64], bf16, name=f"ch_{h}")
                nc.sync.dma_start(t[:], w4[:, h:h + 2])
                halves[h] = t
            quads = {}
            for g in range(3, 8):
                wq = wp.tile([128, 4, C * 64], bf16, name=f"wq_{g}")
                nc.sync.dma_start(wq[:], w4[:, 4 * g:4 * g + 4, :])
                quads[g] = wq

            # scalar queue carries X and the remaining masks in global
            # just-in-time order, in parallel with the sync queue's W stream
            s0a = xpp.tile([128, 1, 13, 36], bf16, name="s0a")
            nc.scalar.dma_start(s0a[:], xp0[:, 0:1])
            mrest = mp.tile([128, 28, 64], bf16, name="mrest")
            nc.scalar.dma_start(mrest[:], mT[:, 4:32])
            s0b1 = xpp.tile([128, 1, 13, 36], bf16, name="s0b1")
            nc.scalar.dma_start(s0b1[:], xp0[:, 1:2])
            s0c2 = xpp.tile([128, 1, 13, 36], bf16, name="s0c2")
            nc.scalar.dma_start(s0c2[:], xp0[:, 2:3])
            s0d = xpp.tile([128, 5, 13, 36], bf16, name="s0d")
            s0d_dma = nc.scalar.dma_start(s0d[:], xp0[:, 3:8])

            # DVE FIFO order matters: critical ops first, then per-chunk
            # mask-muls in consumption order, with each 1-m just ahead of
            # its first use.
            crm = cr[:, 0:256].rearrange("p (c f) -> p c f", c=4)
            nc.vector.tensor_scalar(
                out=crm, in0=crm, scalar1=-1.0, scalar2=1.0,
                op0=mybir.AluOpType.mult, op1=mybir.AluOpType.add,
            )

            def mask_ap(ci):
                if ci < 4:
                    return cr[:, 64 * ci:64 * (ci + 1)]
                return mrest[:, ci - 4, :]

            def mask_mul(w_ap, ci, nch):
                wv = w_ap.rearrange("p (c f) -> p c f", c=nch)
                nc.vector.tensor_tensor(
                    out=wv, in0=wv,
                    in1=mask_ap(ci)[:, None, :].broadcast_to([128, nch, 64]),
                    op=mybir.AluOpType.mult,
                )

            mask_mul(cr[:, 256:384], 0, 2)           # m=0 slice -> first MM
            mask_mul(cr[:, 384:768], 0, 6)           # m=1..3
            mask_mul(cr[:, 768:1280], 0, 8)          # m=4..7
            mask_mul(c1[:], 1, C)
            mask_mul(c23[:, 0, :], 2, C)
            mask_mul(c23[:, 1, :], 3, C)
            nc.vector.tensor_scalar(
                out=mrest[:], in0=mrest[:], scalar1=-1.0, scalar2=1.0,
                op0=mybir.AluOpType.mult, op1=mybir.AluOpType.add,
            )
            for ci in range(4, 12):
                mask_mul(halves[ci - ci % 2][:, ci % 2, :], ci, C)
            for ci in range(12, 32):
                mask_mul(quads[ci // 4][:, ci % 4, :], ci, C)

            def lhsT(ci, m):
                if ci == 0:
                    return cr[:, 256 + 128 * m:384 + 128 * m]
                if ci == 1:
                    return c1[:, 128 * m:128 * (m + 1)]
                if ci < 4:
                    return c23[:, ci - 2, 128 * m:128 * (m + 1)]
                if ci < 12:
                    h = ci - ci % 2
                    return halves[h][:, ci % 2, 128 * m:128 * (m + 1)]
                return quads[ci // 4][:, ci % 4, 128 * m:128 * (m + 1)]

            SLABS = ((0, 13), (11, 24), (22, 34))
            slab_dmas = [None]
            slabs = [None]
            for si, (y0, y1) in list(enumerate(SLABS))[1:]:
                st = xpp.tile([128, y1 - y0, 8, 36], bf16, name=f"slab_{si}")
                slab_dmas.append(nc.scalar.dma_start(st[:], xp[:, y0:y1]))
                slabs.append(st)

            def rhs_ap(n, ci):
                a, sy, sx = CHUNKS[ci]
                lo, hi = _mm_rows(n, sy)
                y0 = 11 * n + 1 - sy - SLABS[n][0]
                x0 = 1 - sx
                if n == 0:
                    t, aa = ((s0a, 0) if a == 0 else
                             (s0b1, 0) if a == 1 else
                             (s0c2, 0) if a == 2 else (s0d, a - 3))
                    return t[:, aa, y0 + lo:y0 + hi, x0:x0 + NV]
                return slabs[n][:, y0 + lo:y0 + hi, a, x0:x0 + NV]

            def ps_ap(ps, n, ci):
                _, sy, _ = CHUNKS[ci]
                lo, hi = _mm_rows(n, sy)
                return ps[:, NV * lo:NV * hi]

            # Phase 2 accumulation order: a full-width (sy=1 when trimming)
            # chunk must open each group so the start=True matmul covers the
            # whole psum width.
            order2 = ([ci for ci in range(32) if CHUNKS[ci][1] == 1]
                      + [ci for ci in range(32) if CHUNKS[ci][1] == 0]) \
                if TRIM else list(range(32))

            # Phase 0 (n=0) is chunk-outer with 8 live psum groups so the PE
            # consumes W chunks as they stream (no all-32-chunks stall).
            pss = [warm_ps] + [pp.tile([128, NT], f32, tag="ps", name=f"ps_0_{i}")
                               for i in range(1, 8)]
            mm0 = {}
            for ci in range(32):
                rhs = rhs_ap(0, ci)
                for m in range(8):
                    mm0[ci, m] = nc.tensor.matmul(
                        ps_ap(pss[m], 0, ci), lhsT(ci, m), rhs,
                        start=(ci == 0), stop=(ci == 31),
                    )
            # Defer the X slabs for phases 1-2 behind PE progress so the
            # startup-critical stream gets the full DMA bandwidth during ramp.
            from concourse.tile_rust import add_dep_helper
            add_dep_helper(s0d_dma.ins, mm0[3, 7].ins,
                           sync=True, reason="late X behind early W stream")
            add_dep_helper(slab_dmas[1].ins, mm0[15, 7].ins,
                           sync=True, reason="slab1 after early phase0")
            add_dep_helper(slab_dmas[2].ins, mm0[23, 7].ins,
                           sync=True, reason="slab2 after mid phase0")
            # per-2m writeback tiles: whole-tile dependency tracking means one
            # big tile would hold every outT DMA until the LAST psum copy.
            for k in range(4):
                ot = op.tile([128, 2, NT], bf16, tag="o", name=f"osb_0_{k}")
                nc.vector.tensor_copy(ot[:, 0, :], pss[2 * k][:])
                nc.vector.tensor_copy(ot[:, 1, :], pss[2 * k + 1][:])
                nc.scalar.dma_start(outT[0, :, 2 * k:2 * k + 2], ot[:])

            batch_groups = []

            def emit_12(m, paired):
                ps1 = pp.tile([128, NT], f32, tag="ps", name=f"ps_12_{m}a")
                ps2 = pp.tile([128, NT], f32, tag="ps", name=f"ps_12_{m}b")
                g1, g2 = [], []
                if paired:
                    for idx, ci in enumerate(order2):
                        st, sp = (idx == 0), (idx == 31)
                        w = lhsT(ci, m)
                        g1.append(nc.tensor.matmul(
                            ps_ap(ps1, 1, ci), w, rhs_ap(1, ci),
                            start=st, stop=sp))
                        g2.append(nc.tensor.matmul(
                            ps_ap(ps2, 2, ci), w, rhs_ap(2, ci),
                            start=st, stop=sp))
                else:
                    for n, g, ps in ((1, g1, ps1), (2, g2, ps2)):
                        for idx, ci in enumerate(order2):
                            g.append(nc.tensor.matmul(
                                ps_ap(ps, n, ci), lhsT(ci, m), rhs_ap(n, ci),
                                start=(idx == 0), stop=(idx == 31)))
                batch_groups.extend([g1, g2])
                return ps1, ps2

            if PAIR:
                # Phases 1+2 fused: m-outer, chunk-mid, n-inner.  The two
                # matmuls of a chunk share lhsT (stationary operand stays
                # loaded) and land in two psum banks.  All X is resident.
                # The last m runs n=1 fully, then n=2, so the n=1 writeback
                # hides under the n=2 matmuls and only the final half-group
                # writeback (split across engines and queues) is exposed.
                for m in range(7):
                    ps1, ps2 = emit_12(m, True)
                    ot = op.tile([128, 2, NT], bf16, tag="o", name=f"osb_12_{m}")
                    nc.vector.tensor_copy(ot[:, 0, :], ps1[:])
                    nc.vector.tensor_copy(ot[:, 1, :], ps2[:])
                    nc.scalar.dma_start(
                        outT[1:3, :, m:m + 1].rearrange("n p o f -> p (n o) f"),
                        ot[:])
                ps1, ps2 = emit_12(7, False)
                o1 = op.tile([128, 1, NT], bf16, tag="o", name="osb_t1")
                nc.vector.tensor_copy(o1[:, 0, :], ps1[:])
                nc.scalar.dma_start(outT[1, :, 7:8], o1[:])
                HNT = NT // 2
                o2 = op.tile([128, 1, NT], bf16, tag="o", name="osb_t2")
                nc.vector.tensor_copy(o2[:, 0, :], ps2[:])
                nc.scalar.dma_start(outT[2, :, 7, 0:HNT], o2[:, 0, 0:HNT])
                nc.sync.dma_start(outT[2, :, 7, HNT:NT], o2[:, 0, HNT:NT])
            else:
                for n in (1, 2):
                    order = order2 if n == 2 else list(range(32))
                    ot = None
                    for m in range(8):
                        ps = pp.tile([128, NT], f32, tag="ps", name=f"ps_{n}_{m}")
                        for idx, ci in enumerate(order):
                            nc.tensor.matmul(
                                ps_ap(ps, n, ci), lhsT(ci, m), rhs_ap(n, ci),
                                start=(idx == 0), stop=(idx == 31),
                            )
                        if n == 2 and m >= 6:
                            o1 = op.tile([128, 1, NT], bf16, tag="o",
                                         name=f"osb_t{m}")
                            nc.vector.tensor_copy(o1[:, 0, :], ps[:])
                            eng = nc.scalar if m == 6 else nc.sync
                            eng.dma_start(outT[n, :, m:m + 1], o1[:])
                        else:
                            if m % 2 == 0:
                                ot = op.tile([128, 2, NT], bf16, tag="o",
                                             name=f"osb_{n}_{m // 2}")
                            nc.vector.tensor_copy(ot[:, m % 2, :], ps[:])
                            if m % 2 == 1:
                                nc.scalar.dma_start(
                                    outT[n, :, m - 1:m + 1], ot[:])

    # Strip the per-MM semaphore increments of phases 1-2: each 32-MM
    # accumulation group increments its PE semaphore once (on the final
    # matmul; walrus requires UpdateValue==1), and every wait threshold on
    # that semaphore is remapped from old cumulative counts to new ones
    # (rounding up to the next kept increment — a waiter that referenced a
    # dropped mid-group count fires at its group end, which is when its
    # data is actually complete).  Phase-0 keeps per-MM increments (the
    # slab/X deferral anchors wait on mid-phase counts there).
    drop = set()
    pe_sem = None
    for grp in batch_groups:
        for g in grp[:-1]:
            drop.add(id(g.ins))
        si = grp[-1].ins.sync_info
        for u in (si.on_update or []):
            if u.sync_type == "semaphore" and u.update_mode == "sem-inc":
                pe_sem = u.id
    # old->new cumulative mapping over the PE-sem update stream (the only
    # updaters are Tensor-engine matmuls, in program order)
    kept_oldcum = []
    oldcum = 0
    all_insts = [i for f in nc.m.functions for bb in f.blocks
                 for i in bb.instructions]
    for inst in all_insts:
        si = getattr(inst, "sync_info", None)
        if not si or not si.on_update:
            continue
        ups, keep = list(si.on_update), []
        changed = False
        for u in ups:
            if (u.sync_type == "semaphore" and u.update_mode == "sem-inc"
                    and u.id == pe_sem):
                oldcum += u.update_value
                if id(inst) in drop:
                    changed = True
                    continue
                kept_oldcum.append(oldcum)
            keep.append(u)
        if changed:
            si.on_update = keep
    import bisect
    for inst in all_insts:
        si = getattr(inst, "sync_info", None)
        if not si or not si.on_wait:
            continue
        ws = list(si.on_wait)
        changed = False
        for w in ws:
            if (w.sync_type == "semaphore" and w.id == pe_sem
                    and w.wait_mode in ("sem-ge-imm", "sem-gte-imm") and w.wait_value):
                j = bisect.bisect_left(kept_oldcum, w.wait_value)
                assert j < len(kept_oldcum), (w.wait_value, len(kept_oldcum))
                w.wait_value = j + 1
                changed = True
        if changed:
            si.on_wait = ws

    _split_multi_sync(nc)
    return nc


def _host_prep(b_ch, mask_b, cos_b):
    """b_ch (16,256,256) f32, mask_b (256,256) f32, cos_b (1024,32,32) f32
    -> dict of device inputs (layout/gather glue only)."""
    bpad = np.pad(b_ch, ((0, 0), (PD, PD), (PD, PD)), mode="edge")
    mpad = np.pad(mask_b, ((PD, PD), (PD, PD)), mode="edge")
    # block layout [bi*33+bj, (c,ry,rx)]
    bT = bpad.reshape(C, 33, 8, 33, 8).transpose(1, 3, 0, 2, 4).reshape(33 * 33, C * 64)
    mTb = mpad.reshape(33, 8, 33, 8).transpose(0, 2, 1, 3).reshape(33 * 33, 64)
    # unfold-as-shifted-block-views: chunk (a,sy,sx), partition p=32*pi+pj
    # reads block row (4a+pi+sy)*33 + (pj+sx).  Pre-gather partition-major.
    pi, pj = np.arange(4)[:, None], np.arange(32)[None, :]
    rows = np.stack([((4 * a + pi + sy) * 33 + (pj + sx)).reshape(128)
                     for (a, sy, sx) in CHUNKS], axis=1)        # [128, 32]
    w4 = np.ascontiguousarray(bT[rows]).astype(ml_dtypes.bfloat16)
    mT = np.ascontiguousarray(mTb[rows]).astype(ml_dtypes.bfloat16)
    xp = np.zeros((1024, 34, 36), np.float32)
    xp[:, 1:33, 1:33] = cos_b
    # [l=128a+p, yy, xx] -> [p, yy, a, xx]; plus an a-major slab-0 copy
    xpb = xp.reshape(8, 128, 34, 36)
    xp0 = np.ascontiguousarray(xpb[:, :, 0:13, :].transpose(1, 0, 2, 3))
    crit0 = np.ascontiguousarray(np.concatenate(
        [mT[:, 0:4].reshape(128, 256), w4[:, 0, :]], axis=1))
    xp = np.ascontiguousarray(xpb.transpose(1, 2, 0, 3))
    return {"crit0": crit0, "w4": w4, "mT": mT,
            "xp": xp.astype(ml_dtypes.bfloat16),
            "xp0": xp0.astype(ml_dtypes.bfloat16)}


def _unshard(outT):
    # outT [3, 128, 8, 11*NV] -> [(c,ry,rx)=128m+p, u=11n+u', v] -> (16,256,256)
    outT = np.asarray(outT).astype(np.float32)
    t = outT.reshape(3, 128, 8, 11, NV).transpose(2, 1, 0, 3, 4).reshape(1024, 33, NV)
    t = t[:, :, :33].reshape(C, 8, 8, 33, 33).transpose(0, 3, 1, 4, 2)
    return t.reshape(C, 264, 264)[:, 4:260, 4:260]


_RUN_KW = {}   # test harness may inject e.g. trace=True
_LAST_RESULTS = [None]
_NC_CACHE = {}


def _get_nc():
    key = (TRIM, PAIR, WARM)
    nc = _NC_CACHE.get(key)
    if nc is None:
        nc = _NC_CACHE[key] = _build_nc()
    return nc


def kernel(cos_similar, b, mask):
    cos_similar = np.ascontiguousarray(np.asarray(cos_similar, dtype=np.float32))
    b = np.ascontiguousarray(np.asarray(b, dtype=np.float32))
    mask = np.ascontiguousarray(np.asarray(mask, dtype=np.float32))

    in_maps = []
    for core in range(N_CORES):
        batch, half = core // 2, core % 2
        ch0 = C * half
        in_maps.append(_host_prep(
            b[batch, ch0:ch0 + C], mask[batch, 0], cos_similar[batch]))

    nc = _get_nc()
    res = run_bass_kernel_spmd(nc, in_maps, list(range(N_CORES)), **_RUN_KW)
    _LAST_RESULTS[0] = res

    out = np.empty((4, 32, 256, 256), np.float32)
    for core in range(N_CORES):
        batch, half = core // 2, core % 2
        ch0 = C * half
        out[batch, ch0:ch0 + C] = _unshard(res.results[core]["outT"])
    return out
